# revision 11
# baseline (speedup 1.0000x reference)
"""ArcFace loss kernel for Trainium2 (8 NeuronCores, vocab-parallel).

Math notes (B=512, D=512, C=100000, S=64, margin M=0.5, label smoothing 0.1):

  cos = normalize(x) @ w          (w columns already unit-norm)
  The margin only changes the label entry of each row: for non-label
  entries cos(arccos(clip(cos))) == clip(cos) == cos (|cos| << 1 here),
  so d_theta == 0 off-label.  Hence
      logits = S*cos               except  logits[i, y_i] = S*cos(theta_i + M)
  Loss per row:  lse_i - (1-eps)*l*_i - (eps/C)*sum_j logits_ij
  with lse = logsumexp over all C classes.

  Since |cos| <= 1, exp(S*cos - S) never overflows, so the sum-exp can use
  the FIXED offset S (=64) instead of a data-dependent row max: no max pass
  and no cross-shard max reduction needed.

Device work per core (classes sharded 8 x 12500):
  cos_shard = x_n^T.T @ w_shard   (fp8e4m3 inputs scaled by 32, DoubleRow
                                   matmuls, fp32 PSUM accumulation)
  per row:  sum_j exp(S*cos - S)   (ACT exp with fused accumulation)
The plain sum_j logits (label-smoothing term) is sum-linear, so it is
computed on the host as x_q . colsum(w_q) from the same quantized values.
Host combines the 8 shards' sum-exp, applies the one-label-per-row margin
correction exactly in float64, and finishes the scalar loss.
"""

import os
import numpy as np
import ml_dtypes
from contextlib import ExitStack

B = 512
D = 512
C = 100000
S = 64.0
MARGIN = 0.5
EPS_SMOOTH = 0.1
N_CORES = 8
CS = C // N_CORES          # 12500 classes per core
KT = 4                     # K subtiles of 128 (D = 512)
MT = 4                     # M tiles of 128 (B = 512)
MMN = 512                  # matmul free dim (exactly one PSUM bank of fp32)

USE_FP8 = os.environ.get("KERNEL_DTYPE", "fp8") == "fp8"
QSCALE = 32.0 if USE_FP8 else 1.0   # pre-scale into fp8's sweet spot

if USE_FP8:
    CHUNK = 2048           # classes per PSUM tile (4 banks)
    # first chunk split small so the first DMA lands fast; tiny chunk last
    # so the post-last-matmul ACT tail is short
    CHUNKS = [512, 1536] + [CHUNK] * 5 + [212]
else:
    CHUNK = 1024
    CHUNKS = [CHUNK] * (CS // CHUNK) + ([CS % CHUNK] if CS % CHUNK else [])
assert sum(CHUNKS) == CS
NCH = len(CHUNKS)

_BUILT = {}


def _build():
    if "nc" in _BUILT:
        return _BUILT["nc"]
    import concourse.tile as tile
    from concourse import bacc, mybir

    in_dt = mybir.dt.float8e4 if USE_FP8 else mybir.dt.bfloat16
    # ACT computes exp(scale*psum + bias); psum holds QSCALE^2 * cos
    act_scale = S / (QSCALE * QSCALE)

    nc = bacc.Bacc("TRN2", target_bir_lowering=False, debug=False,
                   num_devices=N_CORES)

    xt_d = nc.dram_tensor("xt", [128, KT, B], in_dt,
                          kind="ExternalInput").ap()
    w_d = nc.dram_tensor("w", [128, KT, CS], in_dt,
                         kind="ExternalInput").ap()
    se_d = nc.dram_tensor("se", [128, MT * NCH], mybir.dt.float32,
                          kind="ExternalOutput").ap()

    with tile.TileContext(nc) as tc, ExitStack() as ctx:
        const_pool = ctx.enter_context(tc.tile_pool(name="const", bufs=1))
        xt_pool = ctx.enter_context(tc.tile_pool(name="xtp", bufs=1))
        w_pool = ctx.enter_context(tc.tile_pool(name="wp", bufs=4))
        ps_pool = ctx.enter_context(tc.tile_pool(name="psp", bufs=2,
                                                 space="PSUM"))
        out_pool = ctx.enter_context(tc.tile_pool(name="outp", bufs=1))

        # pre-warm: ~4us of dummy matmuls flips the PE HAM to full clock
        # before the first real matmul issues.  The dummy weights are memset
        # on GpSimd (free right after its preamble) so the burst starts as
        # early as possible and bridges seamlessly into the real matmuls.
        warm_w = const_pool.tile([128, 2, 512], in_dt)
        nc.gpsimd.memset(warm_w[:], 1.0)
        warm_ps = ps_pool.tile([128, CHUNK], mybir.dt.float32, tag="ps")
        for _ in range(int(os.environ.get('WARM_MMS', '10'))):
            if USE_FP8:
                nc.tensor.matmul(warm_ps[:, :512], lhsT=warm_w[:, :, :128],
                                 rhs=warm_w[:], start=True, stop=True,
                                 perf_mode=mybir.MatmulPerfMode.DoubleRow)
            else:
                nc.tensor.matmul(warm_ps[:, :512], lhsT=warm_w[:, 0, :128],
                                 rhs=warm_w[:, 0, :], start=True, stop=True)

        bias_t = const_pool.tile([128, 1], mybir.dt.float32)
        nc.vector.memset(bias_t[:], -S)

        # xt on the ACT HWDGE queue so it runs parallel with w chunk 0 on sync
        xt_t = xt_pool.tile([128, KT, B], in_dt)
        nc.scalar.dma_start(xt_t[:], xt_d[:])

        # pre-warm: pull in the exp table while the first DMAs are in flight
        warm_o = const_pool.tile([128, 1], mybir.dt.float32)
        nc.scalar.activation(warm_o[:], bias_t[:],
                             mybir.ActivationFunctionType.Exp,
                             bias=bias_t[:], scale=0.0)

        se_t = out_pool.tile([128, MT * NCH], mybir.dt.float32)

        n0 = 0
        for ci, ncols in enumerate(CHUNKS):
            w_t = w_pool.tile([128, KT, CHUNK], in_dt, tag="w")
            nc.sync.dma_start(w_t[:, :, :ncols], w_d[:, :, n0:n0 + ncols])
            for m in range(MT):
                ps = ps_pool.tile([128, CHUNK], mybir.dt.float32, tag="ps")
                for sub in range(0, ncols, MMN):
                    sn = min(MMN, ncols - sub)
                    if USE_FP8:
                        # DoubleRow: contract 2 K-subtiles (256) per matmul
                        for j in range(KT // 2):
                            nc.tensor.matmul(
                                ps[:, sub:sub + sn],
                                lhsT=xt_t[:, 2 * j:2 * j + 2,
                                          m * 128:(m + 1) * 128],
                                rhs=w_t[:, 2 * j:2 * j + 2, sub:sub + sn],
                                start=(j == 0), stop=(j == KT // 2 - 1),
                                perf_mode=mybir.MatmulPerfMode.DoubleRow,
                            )
                    else:
                        for k in range(KT):
                            nc.tensor.matmul(
                                ps[:, sub:sub + sn],
                                lhsT=xt_t[:, k, m * 128:(m + 1) * 128],
                                rhs=w_t[:, k, sub:sub + sn],
                                start=(k == 0), stop=(k == KT - 1),
                            )
                col = m * NCH + ci
                nc.scalar.activation(
                    ps[:, :ncols], ps[:, :ncols],
                    mybir.ActivationFunctionType.Exp,
                    bias=bias_t[:], scale=act_scale,
                    accum_out=se_t[:, col:col + 1],
                )
            n0 += ncols

        nc.sync.dma_start(se_d[:], se_t[:])

    nc.compile()
    _BUILT["nc"] = nc
    return nc


def _prep_inputs(x, w):
    """Host-side prep: normalize x, quantize, lay out for the device."""
    from concourse import mybir
    np_dt = mybir.dt.np(mybir.dt.float8e4) if USE_FP8 else ml_dtypes.bfloat16

    xn = np.asarray(x, dtype=np.float64)
    xn = xn / np.sqrt((xn * xn).sum(axis=1, keepdims=True))
    xq = (xn * QSCALE).astype(np_dt)                 # [B, D] quantized
    # lhsT layout: [128(d within k-subtile), k, b]
    xt = np.ascontiguousarray(xq.T.reshape(KT, 128, B).transpose(1, 0, 2))

    w32 = np.asarray(w, dtype=np.float32)
    wq = (w32 * np.float32(QSCALE)).astype(np_dt)    # [D, C] quantized
    w_cores = []
    for c in range(N_CORES):
        ws = wq[:, c * CS:(c + 1) * CS]              # [D, CS]
        w_cores.append(np.ascontiguousarray(
            ws.reshape(KT, 128, CS).transpose(1, 0, 2)))  # [128, KT, CS]
    return xq, wq, xt, w_cores


def _run(inputs, trace=False):
    from concourse import bass_utils

    x = np.asarray(inputs["x"])
    y = np.asarray(inputs["y"]).astype(np.int64)
    w = np.asarray(inputs["w"])

    xq, wq, xt, w_cores = _prep_inputs(x, w)
    nc = _build()

    in_maps = [{"xt": xt, "w": w_cores[c]} for c in range(N_CORES)]
    res = bass_utils.run_bass_kernel_spmd(
        nc, in_maps, core_ids=list(range(N_CORES)), trace=trace)

    # ---- host combine (float64, O(B*D + C) work) ----
    s_tot = np.zeros(B)       # sum_j exp(S*cos - S)
    for c in range(N_CORES):
        se = res.results[c]["se"].astype(np.float64)   # [128, MT*NCH]
        # value at [p, m*NCH + ci] belongs to row b = m*128 + p
        s_tot += se.reshape(128, MT, NCH).sum(axis=2).T.reshape(B)

    # de-quantized host-side values, matching what the device multiplied
    xq64 = xq.astype(np.float64) / QSCALE              # [B, D]
    # plain sum of logits is linear: sum_j cos_ij = x_i . colsum(w)
    wsum = wq.astype(np.float32).sum(axis=1, dtype=np.float64) / QSCALE
    c_tot = xq64 @ wsum                                # [B]

    wlab = wq[:, y].astype(np.float64) / QSCALE        # [D, B]
    c_lab = np.einsum("bd,db->b", xq64, wlab)          # cos at label
    l_orig = S * c_lab
    theta = np.arccos(np.clip(c_lab, -1.0 + 1e-05, 1.0 - 1e-05))
    theta_m = np.clip(theta + MARGIN, 1e-05, 3.14159)
    l_star = S * np.cos(theta_m)

    s_adj = s_tot - np.exp(l_orig - S) + np.exp(l_star - S)
    lse = S + np.log(s_adj)
    sum_logits = S * c_tot - l_orig + l_star
    loss_rows = lse - (1.0 - EPS_SMOOTH) * l_star \
        - (EPS_SMOOTH / C) * sum_logits
    loss = np.array(loss_rows.mean(), dtype=np.float32)
    return loss, res


def kernel(**inputs) -> np.ndarray:
    loss, _ = _run(inputs, trace=False)
    return loss


# revision 13
# speedup vs baseline: 1.0170x; 1.0170x over previous
"""ArcFace loss kernel for Trainium2 (8 NeuronCores, vocab-parallel).

Math notes (B=512, D=512, C=100000, S=64, margin M=0.5, label smoothing 0.1):

  cos = normalize(x) @ w          (w columns already unit-norm)
  The margin only changes the label entry of each row: for non-label
  entries cos(arccos(clip(cos))) == clip(cos) == cos (|cos| << 1 here),
  so d_theta == 0 off-label.  Hence
      logits = S*cos               except  logits[i, y_i] = S*cos(theta_i + M)
  Loss per row:  lse_i - (1-eps)*l*_i - (eps/C)*sum_j logits_ij
  with lse = logsumexp over all C classes.

  Since |cos| <= 1, exp(S*cos - S) never overflows, so the sum-exp can use
  the FIXED offset S (=64) instead of a data-dependent row max: no max pass
  and no cross-shard max reduction needed.

Device work per core (classes sharded 8 x 12500):
  cos_shard = x_n^T.T @ w_shard   (fp8e4m3 inputs scaled by 32, DoubleRow
                                   matmuls, fp32 PSUM accumulation)
  per row:  sum_j exp(S*cos - S)   (ACT exp with fused accumulation)
The plain sum_j logits (label-smoothing term) is sum-linear, so it is
computed on the host as x_q . colsum(w_q) from the same quantized values.
Host combines the 8 shards' sum-exp, applies the one-label-per-row margin
correction exactly in float64, and finishes the scalar loss.
"""

import os
import numpy as np
import ml_dtypes
from contextlib import ExitStack

B = 512
D = 512
C = 100000
S = 64.0
MARGIN = 0.5
EPS_SMOOTH = 0.1
N_CORES = 8
CS = C // N_CORES          # 12500 classes per core
KT = 4                     # K subtiles of 128 (D = 512)
MT = 4                     # M tiles of 128 (B = 512)
MMN = 512                  # matmul free dim (exactly one PSUM bank of fp32)

USE_FP8 = os.environ.get("KERNEL_DTYPE", "fp8") == "fp8"
QSCALE = 32.0 if USE_FP8 else 1.0   # pre-scale into fp8's sweet spot

if USE_FP8:
    CHUNK = 2048           # classes per PSUM tile (4 banks)
    # first chunk split small so the first DMA lands fast; tiny chunk last
    # so the post-last-matmul ACT tail is short
    CHUNKS = [512, 1536] + [CHUNK] * 5 + [212]
else:
    CHUNK = 1024
    CHUNKS = [CHUNK] * (CS // CHUNK) + ([CS % CHUNK] if CS % CHUNK else [])
assert sum(CHUNKS) == CS
NCH = len(CHUNKS)

_BUILT = {}


def _build():
    if "nc" in _BUILT:
        return _BUILT["nc"]
    import concourse.tile as tile
    from concourse import bacc, mybir

    in_dt = mybir.dt.float8e4 if USE_FP8 else mybir.dt.bfloat16
    # ACT computes exp(scale*psum + bias); psum holds QSCALE^2 * cos
    act_scale = S / (QSCALE * QSCALE)

    nc = bacc.Bacc("TRN2", target_bir_lowering=False, debug=False,
                   num_devices=N_CORES)

    xt_d = nc.dram_tensor("xt", [128, KT, B], in_dt,
                          kind="ExternalInput").ap()
    w_d = nc.dram_tensor("w", [128, KT, CS], in_dt,
                         kind="ExternalInput").ap()
    se_d = nc.dram_tensor("se", [128, MT * NCH], mybir.dt.float32,
                          kind="ExternalOutput").ap()

    with tile.TileContext(nc) as tc, ExitStack() as ctx:
        const_pool = ctx.enter_context(tc.tile_pool(name="const", bufs=1))
        xt_pool = ctx.enter_context(tc.tile_pool(name="xtp", bufs=1))
        w_pool = ctx.enter_context(tc.tile_pool(name="wp", bufs=4))
        ps_pool = ctx.enter_context(tc.tile_pool(name="psp", bufs=2,
                                                 space="PSUM"))
        out_pool = ctx.enter_context(tc.tile_pool(name="outp", bufs=1))

        # pre-warm: ~4us of dummy matmuls flips the PE HAM to full clock
        # before the first real matmul issues.  The dummy weights are memset
        # on GpSimd (free right after its preamble) so the burst starts as
        # early as possible and bridges seamlessly into the real matmuls.
        warm_w = const_pool.tile([128, 2, 512], in_dt)
        nc.gpsimd.memset(warm_w[:], 1.0)
        warm_ps = ps_pool.tile([128, CHUNK], mybir.dt.float32, tag="ps")
        for _ in range(int(os.environ.get('WARM_MMS', '10'))):
            if USE_FP8:
                nc.tensor.matmul(warm_ps[:, :512], lhsT=warm_w[:, :, :128],
                                 rhs=warm_w[:], start=True, stop=True,
                                 perf_mode=mybir.MatmulPerfMode.DoubleRow)
            else:
                nc.tensor.matmul(warm_ps[:, :512], lhsT=warm_w[:, 0, :128],
                                 rhs=warm_w[:, 0, :], start=True, stop=True)

        # xt on the ACT HWDGE queue so it runs parallel with w chunk 0 on sync
        xt_t = xt_pool.tile([128, KT, B], in_dt)
        nc.scalar.dma_start(xt_t[:], xt_d[:])

        # pre-warm: pull in the exp table while the first DMAs are in flight
        warm_o = const_pool.tile([128, 1], mybir.dt.float32)
        nc.vector.memset(warm_o[:], 0.0)
        nc.scalar.activation(warm_o[:], warm_o[:],
                             mybir.ActivationFunctionType.Exp,
                             bias=0.0, scale=0.0)

        se_t = out_pool.tile([128, MT * NCH], mybir.dt.float32)

        n0 = 0
        for ci, ncols in enumerate(CHUNKS):
            w_t = w_pool.tile([128, KT, CHUNK], in_dt, tag="w")
            nc.sync.dma_start(w_t[:, :, :ncols], w_d[:, :, n0:n0 + ncols])
            for m in range(MT):
                ps = ps_pool.tile([128, CHUNK], mybir.dt.float32, tag="ps")
                for sub in range(0, ncols, MMN):
                    sn = min(MMN, ncols - sub)
                    if USE_FP8:
                        # DoubleRow: contract 2 K-subtiles (256) per matmul
                        for j in range(KT // 2):
                            nc.tensor.matmul(
                                ps[:, sub:sub + sn],
                                lhsT=xt_t[:, 2 * j:2 * j + 2,
                                          m * 128:(m + 1) * 128],
                                rhs=w_t[:, 2 * j:2 * j + 2, sub:sub + sn],
                                start=(j == 0), stop=(j == KT // 2 - 1),
                                perf_mode=mybir.MatmulPerfMode.DoubleRow,
                            )
                    else:
                        for k in range(KT):
                            nc.tensor.matmul(
                                ps[:, sub:sub + sn],
                                lhsT=xt_t[:, k, m * 128:(m + 1) * 128],
                                rhs=w_t[:, k, sub:sub + sn],
                                start=(k == 0), stop=(k == KT - 1),
                            )
                col = m * NCH + ci
                nc.scalar.activation(
                    ps[:, :ncols], ps[:, :ncols],
                    mybir.ActivationFunctionType.Exp,
                    bias=0.0, scale=act_scale,
                    accum_out=se_t[:, col:col + 1],
                )
            n0 += ncols

        nc.sync.dma_start(se_d[:], se_t[:])

    nc.compile()
    _BUILT["nc"] = nc
    return nc


def _prep_inputs(x, w):
    """Host-side prep: normalize x, quantize, lay out for the device."""
    from concourse import mybir
    np_dt = mybir.dt.np(mybir.dt.float8e4) if USE_FP8 else ml_dtypes.bfloat16

    xn = np.asarray(x, dtype=np.float64)
    xn = xn / np.sqrt((xn * xn).sum(axis=1, keepdims=True))
    xq = (xn * QSCALE).astype(np_dt)                 # [B, D] quantized
    # lhsT layout: [128(d within k-subtile), k, b]
    xt = np.ascontiguousarray(xq.T.reshape(KT, 128, B).transpose(1, 0, 2))

    w32 = np.asarray(w, dtype=np.float32)
    wq = (w32 * np.float32(QSCALE)).astype(np_dt)    # [D, C] quantized
    w_cores = []
    for c in range(N_CORES):
        ws = wq[:, c * CS:(c + 1) * CS]              # [D, CS]
        w_cores.append(np.ascontiguousarray(
            ws.reshape(KT, 128, CS).transpose(1, 0, 2)))  # [128, KT, CS]
    return xq, wq, xt, w_cores


def _ensure_hook_stub():
    import sys, types
    if "antenv.axon_hooks" in sys.modules:
        return
    import antenv
    m = types.ModuleType("antenv.axon_hooks")
    m._hook = None
    m.set_axon_ntff_profile_hook = lambda h: setattr(m, "_hook", h)
    m.get_axon_ntff_profile_hook = lambda: m._hook
    sys.modules["antenv.axon_hooks"] = m
    antenv.axon_hooks = m


def _run(inputs, trace=False):
    _ensure_hook_stub()
    from concourse import bass_utils

    x = np.asarray(inputs["x"])
    y = np.asarray(inputs["y"]).astype(np.int64)
    w = np.asarray(inputs["w"])

    xq, wq, xt, w_cores = _prep_inputs(x, w)
    nc = _build()

    in_maps = [{"xt": xt, "w": w_cores[c]} for c in range(N_CORES)]
    res = bass_utils.run_bass_kernel_spmd(
        nc, in_maps, core_ids=list(range(N_CORES)), trace=trace)

    # ---- host combine (float64, O(B*D + C) work) ----
    s_tot = np.zeros(B)       # sum_j exp(S*cos - S)
    for c in range(N_CORES):
        se = res.results[c]["se"].astype(np.float64)   # [128, MT*NCH]
        # value at [p, m*NCH + ci] belongs to row b = m*128 + p
        s_tot += se.reshape(128, MT, NCH).sum(axis=2).T.reshape(B)
    s_tot *= np.exp(-S)

    # de-quantized host-side values, matching what the device multiplied
    xq64 = xq.astype(np.float64) / QSCALE              # [B, D]
    # plain sum of logits is linear: sum_j cos_ij = x_i . colsum(w)
    wsum = wq.astype(np.float32).sum(axis=1, dtype=np.float64) / QSCALE
    c_tot = xq64 @ wsum                                # [B]

    wlab = wq[:, y].astype(np.float64) / QSCALE        # [D, B]
    c_lab = np.einsum("bd,db->b", xq64, wlab)          # cos at label
    l_orig = S * c_lab
    theta = np.arccos(np.clip(c_lab, -1.0 + 1e-05, 1.0 - 1e-05))
    theta_m = np.clip(theta + MARGIN, 1e-05, 3.14159)
    l_star = S * np.cos(theta_m)

    s_adj = s_tot - np.exp(l_orig - S) + np.exp(l_star - S)
    lse = S + np.log(s_adj)
    sum_logits = S * c_tot - l_orig + l_star
    loss_rows = lse - (1.0 - EPS_SMOOTH) * l_star \
        - (EPS_SMOOTH / C) * sum_logits
    loss = np.array(loss_rows.mean(), dtype=np.float32)
    return loss, res


def kernel(**inputs) -> np.ndarray:
    loss, _ = _run(inputs, trace=False)
    return loss
